# revision 6
# baseline (speedup 1.0000x reference)
"""Trainium2 Bass kernel for CausalSelfAttention with lightning (linear)
attention + LRPE, sharded over 8 NeuronCores.

Fused per-head pipeline (v2): for each head, the qk GEMM accumulates in
PSUM and the DVE applies LRPE straight out of PSUM into SBUF-resident
fp16 tiles (qlc/qls/klc/kls); the head's lightning attention is emitted
inline so PE attention work interleaves with the next head's GEMM.
v = x @ W_v.T is computed in two 512-wide half-blocks (spilled to DRAM)
scheduled just before the heads that consume them. Output projection
(phase 3) runs at the end from SBUF-resident yT tiles.

Sharding: 8 cores = (batch 4) x (head-group 2, 8 heads each). Each core
computes a partial output (2048, 2048); host sums the two partials per
batch.
"""
import contextlib
import math

import numpy as np

import concourse.tile as tile
from concourse import bacc, mybir
from concourse import bass_utils

F32 = mybir.dt.float32
F32R = mybir.dt.float32r
F16 = mybir.dt.float16

P = 128
DIM = 2048
HEADS = 16
B = 4
T = 2048
E = DIM // HEADS          # 128
HPC = HEADS // 2          # heads per core = 8
CHUNK = 256               # our chunk size (exact identity holds for any size)
NCH = T // CHUNK          # 8 chunks
KC = DIM // P             # 16 contraction chunks of 128
NT = T // 512             # 4 token tiles of 512
QK_DIMS = 2 * HPC * E     # 2048 (q then k, T-layout)
YD = HPC * E              # 1024 y dims per core

_NC_CACHE = None


def _build_nc(loop_n: int = 1, phases: str = "123"):
    """Build the (SPMD-identical) Bass program for one core.

    loop_n > 1 wraps the compute in a hardware loop (benchmarking only)."""
    nc = bacc.Bacc("TRN2", target_bir_lowering=False, debug=False,
                   enable_asserts=False, num_devices=8)

    xt_d = nc.dram_tensor("xt", (DIM, T), F16, kind="ExternalInput")        # x_b.T
    wt_d = nc.dram_tensor("wt", (DIM, QK_DIMS), F16, kind="ExternalInput")  # W_qk.T
    wv_d = nc.dram_tensor("wv", (DIM, YD), F16, kind="ExternalInput")       # W_v.T
    wp_d = nc.dram_tensor("wp", (YD, DIM), F16, kind="ExternalInput")       # w_proj[:, cols].T
    cos_d = nc.dram_tensor("costab", (YD, T), F16, kind="ExternalInput")
    sin_d = nc.dram_tensor("sintab", (YD, T), F16, kind="ExternalInput")
    mask_d = nc.dram_tensor("maskt", (HPC, 2, P, CHUNK), F16, kind="ExternalInput")
    qdec_d = nc.dram_tensor("qdec", (HPC, P, CHUNK), F32, kind="ExternalInput")
    kdec_d = nc.dram_tensor("kdec", (HPC, 2, P), F32, kind="ExternalInput")
    lamc_d = nc.dram_tensor("lamc", (HPC, P), F32, kind="ExternalInput")
    out_d = nc.dram_tensor("out", (T, DIM), F16, kind="ExternalOutput")

    with tile.TileContext(nc) as tc:
        with (
            tc.tile_pool(name="const", bufs=1) as constp,
            tc.tile_pool(name="dram", bufs=1, space="DRAM") as dram,
            tc.tile_pool(name="py", bufs=1) as py,
        ):
            # ---- constants (small) ----
            ident16 = constp.tile([P, P], F16)
            from concourse.masks import make_identity
            make_identity(nc, ident16)
            kdec_t = constp.tile([P, HPC, 2], F32)
            nc.sync.dma_start(kdec_t, kdec_d.ap().rearrange("h j p -> p h j"))
            lamc_t = constp.tile([P, HPC], F32)
            nc.sync.dma_start(lamc_t, lamc_d.ap().rearrange("h p -> p h"))
            mask_t = constp.tile([P, HPC, 2, CHUNK], F16)
            qdec_t = constp.tile([P, HPC, CHUNK], F32)

            vnd0 = dram.tile([T, 512], F16)
            vnd1 = dram.tile([T, 512], F16)

            ylts = []
            for _yi in range(HPC):
                ylts.append(py.tile([P, T], F16, name=f"ylt{_yi}", tag=f"ylt{_yi}"))

            env = dict(locals())
            loop_cm = tc.For_i(0, loop_n, 1) if loop_n > 1 else contextlib.nullcontext()
            with loop_cm:
                _body(nc, tc, env, phases)

    nc.compile()
    return nc


def _v_half(nc, env, nv):
    """v natural-layout half-block: vnd[nv][:, :] = x_b @ W_v.T[:, nv*512:]"""
    xts = env["xts"]; wvt = env["wvt"]; p1ps = env["p1ps"]; p1w = env["p1w"]
    vnds = env["vnds"]
    for mt in range(T // P):                # 16 token tiles
        ps = p1ps.tile([P, 512], F32, name="psv", tag=f"pq{mt % 2}")
        for kc in range(KC):
            nc.tensor.matmul(ps, xts[kc][:, mt * P:(mt + 1) * P],
                             wvt[:, kc, nv * 512:(nv + 1) * 512],
                             start=(kc == 0), stop=(kc == KC - 1))
        so = p1w.tile([P, 512], F16, tag="p1o")
        nc.any.tensor_copy(out=so, in_=ps)
        nc.scalar.dma_start(vnds[nv][mt * P:(mt + 1) * P], so)


def _body(nc, tc, env, which="123"):
    mult = mybir.AluOpType.mult
    add = mybir.AluOpType.add
    COPY = mybir.ActivationFunctionType.Copy
    xt_d = env["xt_d"]; wt_d = env["wt_d"]; wv_d = env["wv_d"]; wp_d = env["wp_d"]
    cos_d = env["cos_d"]; sin_d = env["sin_d"]
    mask_t = env["mask_t"]; qdec_t = env["qdec_t"]
    kdec_t = env["kdec_t"]; lamc_t = env["lamc_t"]; ident16 = env["ident16"]
    vnds = [env["vnd0"], env["vnd1"]]; out_d = env["out_d"]
    ylts = env["ylts"]

    # ================= fused qkv GEMM + attention =================
    with (
        tc.tile_pool(name="p1x", bufs=1) as p1x,
        tc.tile_pool(name="p1w", bufs=3) as p1w,
        tc.tile_pool(name="p1v", bufs=1) as p1v,
        tc.tile_pool(name="plr", bufs=1) as plr,
        tc.tile_pool(name="ptab", bufs=1) as ptab,
        tc.tile_pool(name="p2io", bufs=2) as p2io,
        tc.tile_pool(name="p2c", bufs=2) as p2c,
        tc.tile_pool(name="p2s", bufs=2) as p2s,
        tc.tile_pool(name="p1ps", bufs=1, space="PSUM") as p1ps,
        tc.tile_pool(name="p2ps", bufs=2, space="PSUM") as p2ps,
        tc.tile_pool(name="p2ps1", bufs=1, space="PSUM") as p2ps1,
    ):
        xt_src = xt_d.ap().rearrange("(kc p) t -> p kc t", p=P)
        xts = []
        for kc in range(KC):
            xts.append(p1x.tile([P, T], F16, name=f"xt{kc}", tag=f"xt{kc}"))
        nc.sync.dma_start(xts[0], xt_src[:, 0])
        wv_src = wv_d.ap().rearrange("(kc p) m -> p kc m", p=P)
        wvt = p1v.tile([P, KC, 1024], F16, tag="wv")
        env2 = dict(env)
        env2.update(xts=xts, wvt=wvt, p1ps=p1ps, p1w=p1w, vnds=vnds)

        wt_src = wt_d.ap().rearrange("(kc p) m -> p kc m", p=P)

        for h in range(HPC):
            # ---- qk GEMM for head h: T-layout with LRPE fused from PSUM ----
            r = h * P
            wms = []
            for qk in range(2):
                m = h + HPC * qk
                wmt = p1w.tile([P, KC, P], F16, name=f"wm{qk}", tag="wm")
                nc.sync.dma_start(wmt, wt_src[:, :, m * P:(m + 1) * P])
                wms.append(wmt)
            cost = ptab.tile([P, T], F16, tag="cost")
            sint = ptab.tile([P, T], F16, tag="sint")
            if h == 0:
                for kc in range(1, 4):
                    nc.sync.dma_start(xts[kc], xt_src[:, kc])
            nc.sync.dma_start(cost, cos_d.ap()[r:r + P])
            nc.sync.dma_start(sint, sin_d.ap()[r:r + P])
            if h == 0:
                for kc in range(4, KC):
                    nc.sync.dma_start(xts[kc], xt_src[:, kc])

            qlc = plr.tile([P, T], F16, tag="qlc")
            qls = plr.tile([P, T], F16, tag="qls")
            klc = plr.tile([P, T], F16, tag="klc")
            kls = plr.tile([P, T], F16, tag="kls")
            for qk, (dc, ds) in ((0, (qlc, qls)), (1, (klc, kls))):
                wm = wms[qk]
                for n in range(NT):             # 4
                    ps = p1ps.tile([P, 512], F32, name=f"psq{n}",
                                   tag=f"pq{n % 2}")
                    for kc in range(KC):            # 16
                        nc.tensor.matmul(ps, wm[:, kc],
                                         xts[kc][:, n * 512:(n + 1) * 512],
                                         start=(kc == 0), stop=(kc == KC - 1))
                    nsl = slice(n * 512, (n + 1) * 512)
                    nc.vector.tensor_tensor(dc[:, nsl], ps, cost[:, nsl], mult)
                    nc.vector.tensor_tensor(ds[:, nsl], ps, sint[:, nsl], mult)

            # v half-blocks: half0 right after head 0's GEMM, half1 after
            # head 3's (heads 4-7 consume it much later)
            if h == 0:
                nc.sync.dma_start(wvt, wv_src)
                mask_d = env["mask_d"]; qdec_d = env["qdec_d"]
                nc.sync.dma_start(mask_t,
                                  mask_d.ap().rearrange("h j p c -> p h j c"))
                nc.sync.dma_start(qdec_t,
                                  qdec_d.ap().rearrange("h p c -> p h c"))
                _v_half(nc, env2, 0)
            elif h == 3:
                _v_half(nc, env2, 1)

            # ---- attention for head h ----
            vna = p2io.tile([P, KC, E], F16, tag="vna")
            rr = (h % 4) * P
            nc.sync.dma_start(
                vna, vnds[h // 4][:, rr:rr + P].rearrange("(tt p) d -> p tt d", p=P))

            st0 = p2s.tile([P, E], F32R, tag="st0")
            st1 = p2s.tile([P, E], F32R, tag="st1")
            ylt = ylts[h]
            lam_col = lamc_t[:, h:h + 1]

            for i in range(NCH):
                sl = slice(i * CHUNK, (i + 1) * CHUNK)
                # k natural layout via PE transpose (fp16) + k_decay scale
                knat = p2c.tile([P, 2, 2 * E], F16, tag="knat")
                for half in range(2):
                    hsl = slice(i * CHUNK + half * P, i * CHUNK + (half + 1) * P)
                    kd = kdec_t[:, h, half:half + 1]
                    pk0 = p2ps.tile([P, P], F16, tag="ptr")
                    nc.tensor.transpose(pk0, klc[:, hsl], ident16)
                    nc.scalar.activation(knat[:, half, 0:E], pk0, COPY,
                                         bias=0.0, scale=kd)
                    pk1 = p2ps.tile([P, P], F16, tag="ptr")
                    nc.tensor.transpose(pk1, kls[:, hsl], ident16)
                    nc.scalar.activation(knat[:, half, E:2 * E], pk1, COPY,
                                         bias=0.0, scale=kd)
                # scoresT (two j-half tiles), mask multiply
                smask = []
                for jh in range(2):
                    jsl = slice(i * CHUNK + jh * P, i * CHUNK + (jh + 1) * P)
                    sps = p2ps.tile([P, CHUNK], F32, tag="sco")
                    nc.tensor.matmul(sps, klc[:, jsl], qlc[:, sl],
                                     start=True, stop=False)
                    nc.tensor.matmul(sps, kls[:, jsl], qls[:, sl],
                                     start=False, stop=True)
                    sm = p2c.tile([P, CHUNK], F16, tag=f"smask{jh}")
                    nc.vector.tensor_tensor(sm, sps, mask_t[:, h, jh], mult)
                    smask.append(sm)
                # oT
                ops = p2ps1.tile([E, CHUNK], F32, tag="ops")
                nc.tensor.matmul(ops, vna[:, 2 * i], smask[0],
                                 start=True, stop=False)
                nc.tensor.matmul(ops, vna[:, 2 * i + 1], smask[1],
                                 start=False, stop=(i == 0))
                if i > 0:
                    qsc = p2c.tile([P, CHUNK], F32R, tag="qsc")
                    qss = p2c.tile([P, CHUNK], F32R, tag="qss")
                    nc.vector.tensor_tensor(qsc, qlc[:, sl], qdec_t[:, h], mult)
                    nc.vector.tensor_tensor(qss, qls[:, sl], qdec_t[:, h], mult)
                    nc.tensor.matmul(ops, st0, qsc, start=False, stop=False)
                    nc.tensor.matmul(ops, st1, qss, start=False, stop=True)
                nc.scalar.copy(ylt[:, sl], ops)
                # state update (not needed after the last chunk)
                if i < NCH - 1:
                    sadd = p2ps1.tile([E, 2, E], F32, tag="sadd")
                    sadd0 = sadd[:, 0]
                    sadd1 = sadd[:, 1]
                    nc.tensor.matmul(sadd0, knat[:, 0, 0:E], vna[:, 2 * i],
                                     start=True, stop=False)
                    nc.tensor.matmul(sadd0, knat[:, 1, 0:E], vna[:, 2 * i + 1],
                                     start=False, stop=True)
                    nc.tensor.matmul(sadd1, knat[:, 0, E:2 * E], vna[:, 2 * i],
                                     start=True, stop=False)
                    nc.tensor.matmul(sadd1, knat[:, 1, E:2 * E], vna[:, 2 * i + 1],
                                     start=False, stop=True)
                    if i == 0:
                        nc.vector.tensor_copy(out=st0, in_=sadd0)
                        nc.vector.tensor_copy(out=st1, in_=sadd1)
                    else:
                        nc.vector.scalar_tensor_tensor(
                            out=st0, in0=st0.bitcast(F32), scalar=lam_col,
                            in1=sadd0, op0=mult, op1=add)
                        nc.vector.scalar_tensor_tensor(
                            out=st1, in0=st1.bitcast(F32), scalar=lam_col,
                            in1=sadd1, op0=mult, op1=add)

    # ================= out = yT.T @ wpT =================
    with (
        tc.tile_pool(name="p3w", bufs=1) as p3w,
        tc.tile_pool(name="p3y", bufs=3) as p3y,
        tc.tile_pool(name="p3ps", bufs=2, space="PSUM") as p3ps,
    ):
        nkc3 = YD // P  # 8
        wp_src = wp_d.ap().rearrange("(kc p) n -> p kc n", p=P)
        wpt = p3w.tile([P, nkc3, DIM], F16, tag="wpt")
        for kc in range(nkc3):
            nc.sync.dma_start(wpt[:, kc], wp_src[:, kc])
        for m in range(T // P):                 # 16
            pss = [p3ps.tile([P, 512], F32, name=f"ps3{n}", tag=f"p3{n}")
                   for n in range(NT)]
            for kc in range(nkc3):
                for n in range(NT):             # 4
                    nc.tensor.matmul(pss[n], ylts[kc][:, m * P:(m + 1) * P],
                                     wpt[:, kc, n * 512:(n + 1) * 512],
                                     start=(kc == 0), stop=(kc == nkc3 - 1))
            for n in range(NT):
                so = p3y.tile([P, 512], F16, tag="p3o")
                nc.any.tensor_copy(out=so, in_=pss[n])
                nc.scalar.dma_start(
                    out_d.ap()[m * P:(m + 1) * P, n * 512:(n + 1) * 512], so)


def _get_nc():
    global _NC_CACHE
    if _NC_CACHE is None:
        _NC_CACHE = _build_nc()
    return _NC_CACHE


def _slopes(h):
    start = 2.0 ** (-(2.0 ** -(math.log2(h) - 3)))
    return np.array([start ** (i + 1) for i in range(h)], dtype=np.float64)


def _prepare_in_maps(x, w_qkv, w_proj, theta):
    slopes = _slopes(HEADS)
    t = np.arange(T, dtype=np.float64)
    idx = np.arange(CHUNK, dtype=np.float64)

    in_maps = []
    for core in range(8):
        b, g = divmod(core, 2)
        heads = np.arange(g * HPC, (g + 1) * HPC)

        xt = np.ascontiguousarray(x[b].T).astype(np.float16)

        qk_rows = np.concatenate([
            np.arange(g * YD, (g + 1) * YD),                 # q rows
            np.arange(DIM + g * YD, DIM + (g + 1) * YD),     # k rows
        ])
        wt = np.ascontiguousarray(w_qkv[qk_rows].T).astype(np.float16)
        v_rows = np.arange(2 * DIM + g * YD, 2 * DIM + (g + 1) * YD)
        wv = np.ascontiguousarray(w_qkv[v_rows].T).astype(np.float16)

        wp = np.ascontiguousarray(w_proj[:, g * YD:(g + 1) * YD].T).astype(np.float16)

        th = theta.reshape(HEADS, E)[heads].astype(np.float64)  # (8, 128)
        ang = th[:, :, None] * t[None, None, :]                 # (8, 128, T)
        costab = np.cos(ang).astype(np.float16).reshape(YD, T)
        sintab = np.sin(ang).astype(np.float16).reshape(YD, T)

        s = slopes[heads]                                       # (8,)
        diff = idx[:, None] - idx[None, :]                      # (i, j)
        maskt = np.where(
            diff[None] >= 0, np.exp(-s[:, None, None] * diff[None]), 0.0
        )                                                       # (8, i, j) = diag_decay
        maskt = np.ascontiguousarray(
            maskt.transpose(0, 2, 1).reshape(HPC, 2, P, CHUNK)).astype(np.float16)
        qdec = np.exp(-s[:, None] * (idx + 1.0)[None]).astype(np.float32)  # (8, 256)
        qdec = np.broadcast_to(qdec[:, None, :], (HPC, P, CHUNK)).copy()
        kdec = np.exp(-s[:, None] * (CHUNK - 1.0 - idx)[None]).astype(np.float32)
        kdec = np.ascontiguousarray(kdec.reshape(HPC, 2, P))
        lamc = np.exp(-s * CHUNK).astype(np.float32)            # (8,)
        lamc = np.broadcast_to(lamc[:, None], (HPC, P)).copy()

        in_maps.append({
            "xt": xt, "wt": wt, "wv": wv, "wp": wp,
            "costab": costab, "sintab": sintab,
            "maskt": maskt, "qdec": qdec, "kdec": kdec, "lamc": lamc,
        })
    return in_maps


def kernel(x, w_qkv, w_proj, theta):
    x = np.asarray(x)
    w_qkv = np.asarray(w_qkv)
    w_proj = np.asarray(w_proj)
    theta = np.asarray(theta)

    nc = _get_nc()
    in_maps = _prepare_in_maps(x, w_qkv, w_proj, theta)
    res = bass_utils.run_bass_kernel_spmd(nc, in_maps, core_ids=list(range(8)))

    out = np.empty((B, T, DIM), dtype=np.float32)
    for b in range(B):
        out[b] = (res.results[2 * b]["out"].astype(np.float32)
                  + res.results[2 * b + 1]["out"].astype(np.float32))
    return out
